# revision 16
# baseline (speedup 1.0000x reference)
# nn_DirectionalConv on TRN2 (8 NeuronCores), Bass/Tile.
#
#   out[r] = deg_inv[r] * sum_{e: row[e]==r} edge_weight[e] * x[col[e]]
#   x: [100000, 32] f32, edge_index: [2, 1600000] i32 (row=dst, col=src)
#
# Strategy (destination-sharded, slot grid + bulk dma_gather):
#  * Host sorts destination rows by degree, packs them into blocks of 128
#    rows, deals blocks to the 8 cores snake-wise. Row (g, p) owns K_sched[g]
#    edge slots in a per-core [128, S] slot grid (column-major flat order
#    i = s*128 + p matches dma_gather's output layout).
#  * x is converted to bf16 and viewed as a quad table [N/4, 128] (one elem
#    = 4 consecutive rows = 256B) so indices fit in int16 (dma_gather's
#    index dtype) and elements satisfy the 256B-multiple constraint.
#  * One dma_gather per span of KSPAN columns fetches 128*ext quads with a
#    single SWDGE instruction (descriptors are generated at ~0.34ns each,
#    vs ~1us fixed cost per indirect_dma_start in the old per-column form).
#  * DVE: multiply gathered quads by per-lane weights (w*deg_inv at the
#    edge's quad phase, 0 elsewhere -> quad-lane select), collapse 4->1 with
#    two adds, then segment-reduce each group's K columns -> [128, 32] f32.
#  * No scatter, no collectives. The host unpermutes rows at the end.
import numpy as np
import ml_dtypes

P = 128
F = 32
QUAD = 4
KSPAN = 64
N_CORES = 8

LAST_EXEC_TIME_NS = None


def _build_schedule(row, col, w, deg_inv, N):
    E = row.shape[0]
    deg = np.bincount(row, minlength=N).astype(np.int64)
    kmax = int(deg.max(initial=0))
    assert kmax <= KSPAN, "node degree exceeds one span"
    B_total = -(-N // (P * N_CORES)) * N_CORES
    N_pad = B_total * P
    deg_pad = np.concatenate([deg, np.zeros(N_pad - N, np.int64)])
    order = np.argsort(deg_pad, kind="stable")
    rank = np.empty(N_pad, np.int64)
    rank[order] = np.arange(N_pad)

    G = B_total // N_CORES
    K_blk = deg_pad[order].reshape(B_total, P).max(axis=1)
    K_sched = K_blk.reshape(G, N_CORES).max(axis=1).astype(np.int64)
    slot_base = np.zeros(G + 1, np.int64)
    acc = 0
    for g in range(G):
        k = int(K_sched[g])
        if k > 0 and (acc % KSPAN) + k > KSPAN:
            acc = -(-acc // KSPAN) * KSPAN
        slot_base[g] = acc
        acc += k
    slot_base[G] = acc
    S_pp = acc
    S_pad = -(-max(S_pp, 1) // KSPAN) * KSPAN

    pr = rank[row]
    blk_e = pr // P
    p_e = (pr % P).astype(np.int64)
    g_e = blk_e // N_CORES
    j_e = blk_e % N_CORES
    core_e = np.where(g_e % 2 == 0, j_e, N_CORES - 1 - j_e)
    if E > 0:
        o = np.argsort(pr, kind="stable")
        pr_s = pr[o]
        first = np.r_[True, pr_s[1:] != pr_s[:-1]]
        run_start = np.maximum.accumulate(np.where(first, np.arange(E), 0))
        k_s = np.arange(E) - run_start
        k_e = np.empty(E, np.int64)
        k_e[o] = k_s
    else:
        k_e = np.zeros(0, np.int64)
    off_e = slot_base[g_e] + k_e

    # int16 quad-index tensor in dma_gather's layout: flat slot i = jloc*128+p
    # lives at [i % 16, span*KSPAN*8 + i//16], replicated x8 down partitions.
    quad = (col >> 2).astype(np.int16)
    phase = (col & 3).astype(np.int64)
    s_e = off_e // KSPAN
    i_e = (off_e % KSPAN) * P + p_e
    idx16 = np.zeros((N_CORES, 16, 8 * S_pad), np.int16)
    idx16[core_e, i_e % 16, s_e * KSPAN * 8 + i_e // 16] = quad
    idx16 = np.tile(idx16, (1, 8, 1))

    wdi = (w.astype(np.float64) * deg_inv[row].astype(np.float64)).astype(np.float32)
    w4 = np.zeros((N_CORES, P, QUAD * S_pad), ml_dtypes.bfloat16)
    w4[core_e, p_e, QUAD * off_e + phase] = wdi.astype(ml_dtypes.bfloat16)

    col_used = np.zeros((N_CORES, S_pad), bool)
    col_used[core_e, off_e] = True
    used_any = col_used.any(axis=0)

    return dict(order=order, K_sched=K_sched, slot_base=slot_base, S_pp=S_pp,
                S_pad=S_pad, G=G, idx16=idx16, w4=w4,
                used_any=used_any, N_pad=N_pad)


def _build_kernel(sched, N, gather_bufs=6):
    import concourse.bass as bass
    import concourse.bacc as bacc
    import concourse.tile as tile
    import concourse.mybir as mybir

    K_sched = sched["K_sched"]
    slot_base = sched["slot_base"]
    S_pad = sched["S_pad"]
    G = sched["G"]
    used_any = sched["used_any"]
    n_spans = S_pad // KSPAN
    NQ = N // QUAD

    nc = bacc.Bacc("TRN2", target_bir_lowering=False, debug=False,
                   num_devices=N_CORES, num_swdge_queues=4)

    x4 = nc.dram_tensor("x4", [NQ, QUAD * F], mybir.dt.bfloat16,
                        kind="ExternalInput")
    idx = nc.dram_tensor("idx", [P, 8 * S_pad], mybir.dt.int16,
                         kind="ExternalInput")
    ws = nc.dram_tensor("ws", [P, QUAD * S_pad], mybir.dt.bfloat16,
                        kind="ExternalInput")
    out = nc.dram_tensor("out", [G * P, F], mybir.dt.float32,
                         kind="ExternalOutput")

    with tile.TileContext(nc) as tc:
        with (
            tc.tile_pool(name="ip", bufs=1) as ip,
            tc.tile_pool(name="wp", bufs=1) as wp,
            tc.tile_pool(name="gp", bufs=gather_bufs) as gp,
            tc.tile_pool(name="cp", bufs=3) as cp,
            tc.tile_pool(name="mp", bufs=3) as mp,
            tc.tile_pool(name="rp", bufs=4) as rp,
        ):
            extents = []
            for s in range(n_spans):
                u = used_any[s * KSPAN:(s + 1) * KSPAN]
                ext = int(np.max(np.nonzero(u)[0]) + 1) if u.any() else 0
                assert u[:ext].all(), "span used columns not a prefix"
                extents.append(ext)

            idx_all = ip.tile([P, 8 * S_pad], mybir.dt.int16)
            nc.sync.dma_start(out=idx_all[:], in_=idx[:])
            w_all = wp.tile([P, QUAD * S_pad], mybir.dt.bfloat16)
            nc.sync.dma_start(out=w_all[:], in_=ws[:])

            msg_tiles = {}
            for s in range(n_spans):
                ext = extents[s]
                if ext == 0:
                    msg_tiles[s] = None
                    continue
                g_t = gp.tile([P, KSPAN, QUAD * F], mybir.dt.bfloat16, tag="g")
                h = (ext + 1) // 2
                halves = [(0, h), (h, ext)] if ext > h else [(0, ext)]
                for hi, (j0, j1) in enumerate(halves):
                    nc.gpsimd.dma_gather(
                        out_ap=g_t[:, j0:j1, :],
                        in_ap=x4[:],
                        idxs_ap=idx_all[:, (s * KSPAN + j0) * 8:(s * KSPAN + j1) * 8],
                        num_idxs=P * (j1 - j0),
                        num_idxs_reg=P * (j1 - j0),
                        elem_size=QUAD * F,
                        single_packet=False,
                        queue_num=(2 * s + hi) % 4,
                    )
                    # per-lane weight (quad-phase select folded in)
                    v4 = g_t[:, j0:j1, :].rearrange("p c (q f) -> p (c q) f", q=QUAD)
                    nc.vector.tensor_tensor(
                        out=v4, in0=v4,
                        in1=w_all[:, QUAD * (s * KSPAN + j0):QUAD * (s * KSPAN + j1)]
                            .to_broadcast([P, QUAD * (j1 - j0), F]),
                        op=mybir.AluOpType.mult)
                # quad collapse 4 -> 2 -> 1
                c_t = cp.tile([P, KSPAN, 2 * F], mybir.dt.bfloat16, tag="c")
                nc.vector.tensor_tensor(
                    out=c_t[:, 0:ext, :],
                    in0=g_t[:, 0:ext, 0:2 * F],
                    in1=g_t[:, 0:ext, 2 * F:QUAD * F],
                    op=mybir.AluOpType.add)
                m_t = mp.tile([P, KSPAN, F], mybir.dt.bfloat16, tag="m")
                nc.vector.tensor_tensor(
                    out=m_t[:, 0:ext, :],
                    in0=c_t[:, 0:ext, 0:F],
                    in1=c_t[:, 0:ext, F:2 * F],
                    op=mybir.AluOpType.add)
                msg_tiles[s] = m_t

            g = 0
            while g < G:
                k = int(K_sched[g])
                if k == 0:
                    ge = g
                    while ge < G and int(K_sched[ge]) == 0:
                        ge += 1
                    rz = rp.tile([P, F], mybir.dt.float32, tag="r")
                    nc.vector.memset(rz[:], 0.0)
                    for gg in range(g, ge):
                        nc.sync.dma_start(out=out[gg * P:(gg + 1) * P, :], in_=rz[:])
                    g = ge
                    continue
                s = int(slot_base[g]) // KSPAN
                ge = g + 1
                while (ge < G and int(K_sched[ge]) == k
                       and int(slot_base[ge]) == int(slot_base[ge - 1]) + k
                       and int(slot_base[ge]) // KSPAN == s):
                    ge += 1
                nrun = ge - g
                j0 = int(slot_base[g]) - s * KSPAN
                m_t = msg_tiles[s]
                src = m_t[:, j0:j0 + nrun * k, :].rearrange(
                    "p (r k) f -> p r f k", k=k)
                r_t = rp.tile([P, nrun * F], mybir.dt.float32, tag="r")
                nc.vector.tensor_reduce(out=r_t[:], in_=src,
                                        axis=mybir.AxisListType.X,
                                        op=mybir.AluOpType.add)
                for i, gg in enumerate(range(g, ge)):
                    nc.sync.dma_start(out=out[gg * P:(gg + 1) * P, :],
                                      in_=r_t[:, i * F:(i + 1) * F])
                g = ge

    nc.compile()
    return nc


def _unshard(sched, core_outs, N):
    G = sched["G"]
    order = sched["order"]
    out = np.zeros((N, F), np.float32)
    g_idx = np.arange(G)
    for c in range(N_CORES):
        j = np.where(g_idx % 2 == 0, c, N_CORES - 1 - c)
        blk = g_idx * N_CORES + j
        ranks = (blk[:, None] * P + np.arange(P)).ravel()
        rows = order[ranks]
        mask = rows < N
        out[rows[mask]] = core_outs[c][mask]
    return out


def kernel(x, edge_index, edge_weight, deg_inv):
    global LAST_EXEC_TIME_NS
    import os
    from concourse.bass_utils import run_bass_kernel_spmd

    x = np.ascontiguousarray(np.asarray(x, dtype=np.float32))
    edge_index = np.asarray(edge_index, dtype=np.int32)
    edge_weight = np.asarray(edge_weight, dtype=np.float32)
    deg_inv = np.asarray(deg_inv, dtype=np.float32)
    N = x.shape[0]
    assert N % QUAD == 0

    x4 = np.ascontiguousarray(
        x.astype(ml_dtypes.bfloat16).reshape(N // QUAD, QUAD * F))

    sched = _build_schedule(edge_index[0], edge_index[1], edge_weight, deg_inv, N)
    nc = _build_kernel(sched, N)
    in_maps = [{"x4": x4, "idx": sched["idx16"][c], "ws": sched["w4"][c]}
               for c in range(N_CORES)]

    trace = bool(int(os.environ.get("KERNEL_TRACE", "0")))
    res = run_bass_kernel_spmd(nc, in_maps, core_ids=list(range(N_CORES)),
                               trace=trace)
    if trace:
        LAST_EXEC_TIME_NS = res.exec_time_ns
    return _unshard(sched, [r["out"] for r in res.results], N)


# revision 17
# speedup vs baseline: 1.1412x; 1.1412x over previous
# nn_DirectionalConv on TRN2 (8 NeuronCores), Bass/Tile.
#
#   out[r] = deg_inv[r] * sum_{e: row[e]==r} edge_weight[e] * x[col[e]]
#   x: [100000, 32] f32, edge_index: [2, 1600000] i32 (row=dst, col=src)
#
# Strategy (destination-sharded, slot grid + bulk dma_gather):
#  * Host sorts destination rows by degree, packs them into blocks of 128
#    rows, deals blocks to the 8 cores snake-wise. Row (g, p) owns K_sched[g]
#    edge slots in a per-core [128, S] slot grid (column-major flat order
#    i = s*128 + p matches dma_gather's output layout).
#  * x is converted to bf16 and viewed as a quad table [N/4, 128] (one elem
#    = 4 consecutive rows = 256B) so indices fit in int16 (dma_gather's
#    index dtype) and elements satisfy the 256B-multiple constraint.
#  * One dma_gather per span of KSPAN columns fetches 128*ext quads with a
#    single SWDGE instruction (descriptors are generated at ~0.34ns each,
#    vs ~1us fixed cost per indirect_dma_start in the old per-column form).
#  * DVE: multiply gathered quads by per-lane weights (w*deg_inv at the
#    edge's quad phase, 0 elsewhere -> quad-lane select), collapse 4->1 with
#    two adds, then segment-reduce each group's K columns -> [128, 32] f32.
#  * No scatter, no collectives. The host unpermutes rows at the end.
import numpy as np
import ml_dtypes

P = 128
F = 32
QUAD = 4
KSPAN = 64
N_CORES = 8

LAST_EXEC_TIME_NS = None


def _build_schedule(row, col, w, deg_inv, N):
    E = row.shape[0]
    deg = np.bincount(row, minlength=N).astype(np.int64)
    kmax = int(deg.max(initial=0))
    assert kmax <= KSPAN, "node degree exceeds one span"
    B_total = -(-N // (P * N_CORES)) * N_CORES
    N_pad = B_total * P
    deg_pad = np.concatenate([deg, np.zeros(N_pad - N, np.int64)])
    order = np.argsort(deg_pad, kind="stable")
    rank = np.empty(N_pad, np.int64)
    rank[order] = np.arange(N_pad)

    G = B_total // N_CORES
    K_blk = deg_pad[order].reshape(B_total, P).max(axis=1)
    K_sched = K_blk.reshape(G, N_CORES).max(axis=1).astype(np.int64)
    slot_base = np.zeros(G + 1, np.int64)
    acc = 0
    for g in range(G):
        k = int(K_sched[g])
        if k > 0 and (acc % KSPAN) + k > KSPAN:
            acc = -(-acc // KSPAN) * KSPAN
        slot_base[g] = acc
        acc += k
    slot_base[G] = acc
    S_pp = acc
    S_pad = -(-max(S_pp, 1) // KSPAN) * KSPAN

    pr = rank[row]
    blk_e = pr // P
    p_e = (pr % P).astype(np.int64)
    g_e = blk_e // N_CORES
    j_e = blk_e % N_CORES
    core_e = np.where(g_e % 2 == 0, j_e, N_CORES - 1 - j_e)
    if E > 0:
        o = np.argsort(pr, kind="stable")
        pr_s = pr[o]
        first = np.r_[True, pr_s[1:] != pr_s[:-1]]
        run_start = np.maximum.accumulate(np.where(first, np.arange(E), 0))
        k_s = np.arange(E) - run_start
        k_e = np.empty(E, np.int64)
        k_e[o] = k_s
    else:
        k_e = np.zeros(0, np.int64)
    off_e = slot_base[g_e] + k_e

    # int16 quad-index tensor in dma_gather's layout: flat slot i = jloc*128+p
    # lives at [i % 16, span*KSPAN*8 + i//16], replicated x8 down partitions.
    quad = (col >> 2).astype(np.int16)
    phase = (col & 3).astype(np.int64)
    s_e = off_e // KSPAN
    i_e = (off_e % KSPAN) * P + p_e
    idx16 = np.zeros((N_CORES, 16, 8 * S_pad), np.int16)
    idx16[core_e, i_e % 16, s_e * KSPAN * 8 + i_e // 16] = quad
    idx16 = np.tile(idx16, (1, 8, 1))

    wdi = (w.astype(np.float64) * deg_inv[row].astype(np.float64)).astype(np.float32)
    w4 = np.zeros((N_CORES, P, QUAD * S_pad), ml_dtypes.bfloat16)
    w4[core_e, p_e, QUAD * off_e + phase] = wdi.astype(ml_dtypes.bfloat16)

    col_used = np.zeros((N_CORES, S_pad), bool)
    col_used[core_e, off_e] = True
    used_any = col_used.any(axis=0)

    return dict(order=order, K_sched=K_sched, slot_base=slot_base, S_pp=S_pp,
                S_pad=S_pad, G=G, idx16=idx16, w4=w4,
                used_any=used_any, N_pad=N_pad)


def _build_kernel(sched, N, gather_bufs=6):
    import concourse.bass as bass
    import concourse.bacc as bacc
    import concourse.tile as tile
    import concourse.mybir as mybir

    K_sched = sched["K_sched"]
    slot_base = sched["slot_base"]
    S_pad = sched["S_pad"]
    G = sched["G"]
    used_any = sched["used_any"]
    n_spans = S_pad // KSPAN
    NQ = N // QUAD

    nc = bacc.Bacc("TRN2", target_bir_lowering=False, debug=False,
                   num_devices=N_CORES, num_swdge_queues=4,
                   dynamic_dma_scratch_size=32768)

    x4 = nc.dram_tensor("x4", [NQ, QUAD * F], mybir.dt.bfloat16,
                        kind="ExternalInput")
    idx = nc.dram_tensor("idx", [P, 8 * S_pad], mybir.dt.int16,
                         kind="ExternalInput")
    ws = nc.dram_tensor("ws", [P, QUAD * S_pad], mybir.dt.bfloat16,
                        kind="ExternalInput")
    out = nc.dram_tensor("out", [G * P, F], mybir.dt.float32,
                         kind="ExternalOutput")

    with tile.TileContext(nc) as tc:
        with (
            tc.tile_pool(name="ip", bufs=1) as ip,
            tc.tile_pool(name="wp", bufs=1) as wp,
            tc.tile_pool(name="gp", bufs=gather_bufs) as gp,
            tc.tile_pool(name="cp", bufs=3) as cp,
            tc.tile_pool(name="mp", bufs=3) as mp,
            tc.tile_pool(name="rp", bufs=4) as rp,
        ):
            extents = []
            for s in range(n_spans):
                u = used_any[s * KSPAN:(s + 1) * KSPAN]
                ext = int(np.max(np.nonzero(u)[0]) + 1) if u.any() else 0
                assert u[:ext].all(), "span used columns not a prefix"
                extents.append(ext)

            idx_all = ip.tile([P, 8 * S_pad], mybir.dt.int16)
            nc.sync.dma_start(out=idx_all[:], in_=idx[:])
            w_all = wp.tile([P, QUAD * S_pad], mybir.dt.bfloat16)
            nc.sync.dma_start(out=w_all[:], in_=ws[:])

            msg_tiles = {}
            for s in range(n_spans):
                ext = extents[s]
                if ext == 0:
                    msg_tiles[s] = None
                    continue
                g_t = gp.tile([P, KSPAN, QUAD * F], mybir.dt.bfloat16, tag="g")
                h = (ext + 1) // 2
                halves = [(0, h), (h, ext)] if ext > h else [(0, ext)]
                for hi, (j0, j1) in enumerate(halves):
                    nc.gpsimd.dma_gather(
                        out_ap=g_t[:, j0:j1, :],
                        in_ap=x4[:],
                        idxs_ap=idx_all[:, (s * KSPAN + j0) * 8:(s * KSPAN + j1) * 8],
                        num_idxs=P * (j1 - j0),
                        num_idxs_reg=P * (j1 - j0),
                        elem_size=QUAD * F,
                        single_packet=False,
                        queue_num=1 + (2 * s + hi) % 3,
                    )
                    # per-lane weight (quad-phase select folded in)
                    v4 = g_t[:, j0:j1, :].rearrange("p c (q f) -> p (c q) f", q=QUAD)
                    nc.vector.tensor_tensor(
                        out=v4, in0=v4,
                        in1=w_all[:, QUAD * (s * KSPAN + j0):QUAD * (s * KSPAN + j1)]
                            .to_broadcast([P, QUAD * (j1 - j0), F]),
                        op=mybir.AluOpType.mult)
                # quad collapse 4 -> 2 -> 1
                c_t = cp.tile([P, KSPAN, 2 * F], mybir.dt.bfloat16, tag="c")
                nc.vector.tensor_tensor(
                    out=c_t[:, 0:ext, :],
                    in0=g_t[:, 0:ext, 0:2 * F],
                    in1=g_t[:, 0:ext, 2 * F:QUAD * F],
                    op=mybir.AluOpType.add)
                m_t = mp.tile([P, KSPAN, F], mybir.dt.bfloat16, tag="m")
                nc.vector.tensor_tensor(
                    out=m_t[:, 0:ext, :],
                    in0=c_t[:, 0:ext, 0:F],
                    in1=c_t[:, 0:ext, F:2 * F],
                    op=mybir.AluOpType.add)
                msg_tiles[s] = m_t

            g = 0
            while g < G:
                k = int(K_sched[g])
                if k == 0:
                    ge = g
                    while ge < G and int(K_sched[ge]) == 0:
                        ge += 1
                    rz = rp.tile([P, F], mybir.dt.float32, tag="r")
                    nc.vector.memset(rz[:], 0.0)
                    for gg in range(g, ge):
                        nc.sync.dma_start(out=out[gg * P:(gg + 1) * P, :], in_=rz[:])
                    g = ge
                    continue
                s = int(slot_base[g]) // KSPAN
                ge = g + 1
                while (ge < G and int(K_sched[ge]) == k
                       and int(slot_base[ge]) == int(slot_base[ge - 1]) + k
                       and int(slot_base[ge]) // KSPAN == s):
                    ge += 1
                nrun = ge - g
                j0 = int(slot_base[g]) - s * KSPAN
                m_t = msg_tiles[s]
                src = m_t[:, j0:j0 + nrun * k, :].rearrange(
                    "p (r k) f -> p r f k", k=k)
                r_t = rp.tile([P, nrun * F], mybir.dt.float32, tag="r")
                nc.vector.tensor_reduce(out=r_t[:], in_=src,
                                        axis=mybir.AxisListType.X,
                                        op=mybir.AluOpType.add)
                for i, gg in enumerate(range(g, ge)):
                    nc.sync.dma_start(out=out[gg * P:(gg + 1) * P, :],
                                      in_=r_t[:, i * F:(i + 1) * F])
                g = ge

    nc.compile()
    return nc


def _unshard(sched, core_outs, N):
    G = sched["G"]
    order = sched["order"]
    out = np.zeros((N, F), np.float32)
    g_idx = np.arange(G)
    for c in range(N_CORES):
        j = np.where(g_idx % 2 == 0, c, N_CORES - 1 - c)
        blk = g_idx * N_CORES + j
        ranks = (blk[:, None] * P + np.arange(P)).ravel()
        rows = order[ranks]
        mask = rows < N
        out[rows[mask]] = core_outs[c][mask]
    return out


def kernel(x, edge_index, edge_weight, deg_inv):
    global LAST_EXEC_TIME_NS
    import os
    from concourse.bass_utils import run_bass_kernel_spmd

    x = np.ascontiguousarray(np.asarray(x, dtype=np.float32))
    edge_index = np.asarray(edge_index, dtype=np.int32)
    edge_weight = np.asarray(edge_weight, dtype=np.float32)
    deg_inv = np.asarray(deg_inv, dtype=np.float32)
    N = x.shape[0]
    assert N % QUAD == 0

    x4 = np.ascontiguousarray(
        x.astype(ml_dtypes.bfloat16).reshape(N // QUAD, QUAD * F))

    sched = _build_schedule(edge_index[0], edge_index[1], edge_weight, deg_inv, N)
    nc = _build_kernel(sched, N)
    in_maps = [{"x4": x4, "idx": sched["idx16"][c], "ws": sched["w4"][c]}
               for c in range(N_CORES)]

    trace = bool(int(os.environ.get("KERNEL_TRACE", "0")))
    res = run_bass_kernel_spmd(nc, in_maps, core_ids=list(range(N_CORES)),
                               trace=trace)
    if trace:
        LAST_EXEC_TIME_NS = res.exec_time_ns
    return _unshard(sched, [r["out"] for r in res.results], N)
